# revision 2
# baseline (speedup 1.0000x reference)
"""Multi-head attention (D=2048, H=16, B=2, S=2048, causal, RoPE) on 8 TRN2 cores.

Sharding: tensor-parallel over heads -- 2 heads per core, both batches.
Each core computes q/k/v projections for its 2 heads, RoPE, causal flash-style
attention, and a partial output projection over its heads' columns of wo.
The host sums the 8 partial outputs (the out-projection contracts over heads,
which is the sharded axis).

Self-contained: hardcodes all shapes; only needs numpy/ml_dtypes/concourse.
"""
import os
import sys
import time

for _p in ("/opt/trn_rl_repo",):
    if os.path.isdir(_p) and _p not in sys.path:
        sys.path.append(_p)

import numpy as np
import ml_dtypes
from contextlib import ExitStack

import concourse.bass as bass
import concourse.tile as tile
from concourse import bacc, mybir

BF = mybir.dt.bfloat16
F32 = mybir.dt.float32
BF_NP = ml_dtypes.bfloat16

B = 2
S = 2048
D = 2048
H = 16
HD = 128  # head dim
N_CORES = 8
H_CORE = H // N_CORES          # heads per core = 2
E = H_CORE * HD                # per-core q/k/v width = 256
BS = B * S                     # 4096 flattened tokens
P = 128
SC = 512                       # s-chunk (free dim of projection matmuls)
N_SC = BS // SC                # 8 s-chunks
N_DT = D // P                  # 16 d-tiles (contraction)
QC = 512                       # q-chunk in attention
N_QC = S // QC                 # 4 q-chunks per (batch, head)
N_KT = S // P                  # 16 k-tiles per (batch, head)
SCALE = 1.0 / float(np.sqrt(HD))
ROPE_BASE = 10000.0


def _build_program():
    """Build the per-core Bass program (identical on all cores; data differs)."""
    nc = bacc.Bacc("TRN2", target_bir_lowering=False, debug=False)

    xt_d = nc.dram_tensor("xt", [D, BS], BF, kind="ExternalInput").ap()
    wqt_d = nc.dram_tensor("wqt", [D, E], BF, kind="ExternalInput").ap()
    wkt_d = nc.dram_tensor("wkt", [D, E], BF, kind="ExternalInput").ap()
    wvt_d = nc.dram_tensor("wvt", [D, E], BF, kind="ExternalInput").ap()
    wot_d = nc.dram_tensor("wot", [E, D], BF, kind="ExternalInput").ap()
    cos_d = nc.dram_tensor("cos", [P, S], BF, kind="ExternalInput").ap()
    sin_d = nc.dram_tensor("sin", [P, S], BF, kind="ExternalInput").ap()
    rmat_d = nc.dram_tensor("rmat", [P, P], BF, kind="ExternalInput").ap()
    tri_d = nc.dram_tensor("tri", [P, P], BF, kind="ExternalInput").ap()
    out_d = nc.dram_tensor("out", [BS, D], F32, kind="ExternalOutput").ap()

    with tile.TileContext(nc) as tc:
        with ExitStack() as ctx:
            _emit(ctx, tc, nc, xt_d, wqt_d, wkt_d, wvt_d, wot_d,
                  cos_d, sin_d, rmat_d, tri_d, out_d)
    nc.compile()
    return nc


def _emit(ctx, tc, nc, xt_d, wqt_d, wkt_d, wvt_d, wot_d,
          cos_d, sin_d, rmat_d, tri_d, out_d):
    Exp = mybir.ActivationFunctionType.Exp

    const = ctx.enter_context(tc.tile_pool(name="const", bufs=1))
    xpool = ctx.enter_context(tc.tile_pool(name="xpool", bufs=2))
    qkv = ctx.enter_context(tc.tile_pool(name="qkv", bufs=1))
    rope = ctx.enter_context(tc.tile_pool(name="rope", bufs=4))
    att = ctx.enter_context(tc.tile_pool(name="att", bufs=6))
    nrm = ctx.enter_context(tc.tile_pool(name="nrm", bufs=3))
    outp = ctx.enter_context(tc.tile_pool(name="outp", bufs=3))
    psum = ctx.enter_context(tc.tile_pool(name="psum", bufs=8, space="PSUM"))

    # ---- constants / weights in SBUF ----
    # w?t layout: [128 (d within tile), N_DT * E] bf16, d-tile major on free dim
    wq_sb = const.tile([P, N_DT * E], BF)
    wk_sb = const.tile([P, N_DT * E], BF)
    wv_sb = const.tile([P, N_DT * E], BF)
    for w_sb, w_d in ((wq_sb, wqt_d), (wk_sb, wkt_d), (wv_sb, wvt_d)):
        nc.sync.dma_start(w_sb[:].rearrange("p (t e) -> p t e", e=E),
                          w_d.rearrange("(t p) e -> p t e", p=P))
    # wot: [E, D] -> [128, 2 * D], d'-tile (== head) major
    wo_sb = const.tile([P, H_CORE * D], BF)
    nc.sync.dma_start(wo_sb[:].rearrange("p (t e) -> p t e", e=D),
                      wot_d.rearrange("(t p) e -> p t e", p=P))
    cos_sb = const.tile([P, S], BF)
    sin_sb = const.tile([P, S], BF)
    nc.sync.dma_start(cos_sb[:], cos_d[:])
    nc.sync.dma_start(sin_sb[:], sin_d[:])
    rmat_sb = const.tile([P, P], BF)
    tri_sb = const.tile([P, P], BF)
    nc.sync.dma_start(rmat_sb[:], rmat_d[:])
    nc.sync.dma_start(tri_sb[:], tri_d[:])
    ones_sb = const.tile([P, P], BF)
    nc.vector.memset(ones_sb[:], 1.0)

    # persistent activations
    qT = qkv.tile([P, H_CORE * BS], BF)   # [d, (head, b*s)] rope'd q
    kT = qkv.tile([P, H_CORE * BS], BF)   # [d, (head, b*s)] rope'd k
    v_sb = qkv.tile([P, (BS // P) * E], BF)  # [s within tile, (s-tile, e)]
    aoT = qkv.tile([P, H_CORE * BS], BF)  # [d, (b, head, q)] normalized attn out

    # ---- phase 1: projections + RoPE ----
    for sc in range(N_SC):
        b = sc // (N_SC // B)
        s_lo = (sc % (N_SC // B)) * SC  # within-batch s offset
        xt_c = xpool.tile([P, N_DT * SC], BF, tag="xt")
        for t in range(N_DT):
            nc.sync.dma_start(
                xt_c[:, t * SC:(t + 1) * SC],
                xt_d[t * P:(t + 1) * P, sc * SC:(sc + 1) * SC])

        # qT / kT (with RoPE) per head (e-tile == head)
        for w_sb, dstT in ((wq_sb, qT), (wk_sb, kT)):
            for h in range(H_CORE):
                pp = psum.tile([P, SC], F32, tag="ps")
                for t in range(N_DT):
                    nc.tensor.matmul(
                        pp[:],
                        w_sb[:, t * E + h * HD: t * E + h * HD + HD],
                        xt_c[:, t * SC:(t + 1) * SC],
                        start=(t == 0), stop=(t == N_DT - 1))
                raw = rope.tile([P, SC], BF, tag="raw")
                nc.scalar.copy(raw[:], pp[:])
                rot = psum.tile([P, SC], F32, tag="ps")
                nc.tensor.matmul(rot[:], rmat_sb[:], raw[:], start=True, stop=True)
                t1 = rope.tile([P, SC], BF, tag="t1")
                nc.vector.tensor_mul(t1[:], raw[:], cos_sb[:, s_lo:s_lo + SC])
                t2 = rope.tile([P, SC], BF, tag="t2")
                nc.vector.tensor_mul(t2[:], rot[:], sin_sb[:, s_lo:s_lo + SC])
                dst = dstT[:, h * BS + sc * SC: h * BS + (sc + 1) * SC]
                nc.vector.tensor_add(dst, t1[:], t2[:])

        # v for this s-chunk: 4 s-subtiles of 128
        for st in range(SC // P):
            pv = psum.tile([P, SC], F32, tag="ps")
            for t in range(N_DT):
                nc.tensor.matmul(
                    pv[:, :E],
                    xt_c[:, t * SC + st * P: t * SC + (st + 1) * P],
                    wv_sb[:, t * E:(t + 1) * E],
                    start=(t == 0), stop=(t == N_DT - 1))
            g_st = sc * (SC // P) + st  # global s-tile index 0..31
            nc.scalar.copy(v_sb[:, g_st * E:(g_st + 1) * E], pv[:, :E])

    # ---- phase 2: attention per (batch, head) ----
    for b in range(B):
        for h in range(H_CORE):
            qk_off = h * BS + b * S  # column offset into qT/kT
            for qc in range(N_QC):
                out_ps = psum.tile([P, QC], F32, tag="ps")
                den_ps = psum.tile([P, QC], F32, tag="ps")
                nkt = (qc + 1) * (QC // P)
                for j in range(nkt):
                    di = j - qc * (QC // P)  # >=0 on diagonal tiles
                    q0 = max(di, 0) * P      # valid q suffix start in chunk
                    sc_ps = psum.tile([P, QC], F32, tag="ps")
                    nc.tensor.matmul(
                        sc_ps[:, q0:QC],
                        kT[:, qk_off + j * P: qk_off + (j + 1) * P],
                        qT[:, qk_off + qc * QC + q0: qk_off + (qc + 1) * QC],
                        start=True, stop=True)
                    at = att.tile([P, QC], BF, tag="at")
                    nc.scalar.activation(at[:, q0:QC], sc_ps[:, q0:QC], Exp,
                                         scale=SCALE)
                    if di >= 0:
                        nc.vector.tensor_mul(at[:, q0:q0 + P],
                                             at[:, q0:q0 + P], tri_sb[:])
                    g_st = b * (S // P) + j
                    nc.tensor.matmul(
                        out_ps[:, q0:QC],
                        v_sb[:, g_st * E + h * HD: g_st * E + (h + 1) * HD],
                        at[:, q0:QC],
                        start=(j == 0), stop=(j == nkt - 1))
                    nc.tensor.matmul(
                        den_ps[:, q0:QC],
                        ones_sb[:],
                        at[:, q0:QC],
                        start=(j == 0), stop=(j == nkt - 1))
                rec = nrm.tile([P, QC], F32, tag="rec")
                nc.vector.reciprocal_approx_fast(rec[:], den_ps[:])
                dst = aoT[:, (b * H_CORE + h) * S + qc * QC:
                          (b * H_CORE + h) * S + (qc + 1) * QC]
                nc.vector.tensor_mul(dst, out_ps[:], rec[:])

    # ---- phase 3: partial output projection ----
    for b in range(B):
        for st in range(S // P):
            for ec in range(D // SC):
                po = psum.tile([P, SC], F32, tag="ps")
                for h in range(H_CORE):
                    lhsT = aoT[:, (b * H_CORE + h) * S + st * P:
                               (b * H_CORE + h) * S + (st + 1) * P]
                    nc.tensor.matmul(
                        po[:],
                        lhsT,
                        wo_sb[:, h * D + ec * SC: h * D + (ec + 1) * SC],
                        start=(h == 0), stop=(h == H_CORE - 1))
                o_sb = outp.tile([P, SC], F32, tag="o")
                nc.scalar.copy(o_sb[:], po[:])
                nc.sync.dma_start(
                    out_d[b * S + st * P: b * S + (st + 1) * P,
                          ec * SC:(ec + 1) * SC],
                    o_sb[:])


def _rope_tables():
    """cos/sin tables exactly matching the reference's indexing quirk."""
    inv_freq = (1.0 / (ROPE_BASE ** (np.arange(0, HD, 2, dtype=np.float32) / HD)))
    t = np.arange(S, dtype=np.float32)
    freqs = np.outer(t, inv_freq)                       # [S, 64]
    emb = np.concatenate([freqs, freqs], axis=1)        # [S, 128]
    cos_part = np.cos(emb)[:, ::2]                      # [S, 64]
    sin_part = np.sin(emb)[:, 1::2]                     # [S, 64]
    # COS[d, s] = cos_part[s, d // 2]
    cos = cos_part.T[np.repeat(np.arange(HD // 2), 2)]  # [128, S]
    sin = sin_part.T[np.repeat(np.arange(HD // 2), 2)]
    return np.ascontiguousarray(cos), np.ascontiguousarray(sin)


def _host_prep(x, wq, wk, wv, wo):
    """Build the per-core input maps."""
    bf = BF_NP
    xt = np.ascontiguousarray(
        x.reshape(BS, D).T.astype(bf))                  # [D, BS]
    cos, sin = _rope_tables()
    cos = cos.astype(bf)
    sin = sin.astype(bf)
    rmat = np.zeros((P, P), dtype=np.float32)           # R^T for rot = R @ q
    idx = np.arange(0, P, 2)
    rmat[idx + 1, idx] = -1.0                           # R^T[2j+1, 2j] = -1
    rmat[idx, idx + 1] = 1.0                            # R^T[2j, 2j+1] = +1
    rmat = rmat.astype(bf)
    tri = np.triu(np.ones((P, P), dtype=np.float32)).astype(bf)

    in_maps = []
    for c in range(N_CORES):
        lo, hi = c * E, (c + 1) * E
        in_maps.append({
            "xt": xt,
            "wqt": np.ascontiguousarray(wq[lo:hi].T.astype(bf)),
            "wkt": np.ascontiguousarray(wk[lo:hi].T.astype(bf)),
            "wvt": np.ascontiguousarray(wv[lo:hi].T.astype(bf)),
            "wot": np.ascontiguousarray(wo[:, lo:hi].T.astype(bf)),
            "cos": cos,
            "sin": sin,
            "rmat": rmat,
            "tri": tri,
        })
    return in_maps


_CACHE = {}


def _get_program():
    if "nc" not in _CACHE:
        _CACHE["nc"] = _build_program()
    return _CACHE["nc"]


def _run(in_maps):
    from concourse.bass_utils import run_bass_kernel_spmd
    nc = _get_program()
    res = run_bass_kernel_spmd(nc, in_maps, core_ids=list(range(N_CORES)))
    return res


def kernel(x, wq, wk, wv, wo, attn_mask=None, **_):
    x = np.asarray(x, dtype=np.float32)
    in_maps = _host_prep(np.asarray(x, np.float32), np.asarray(wq, np.float32),
                         np.asarray(wk, np.float32), np.asarray(wv, np.float32),
                         np.asarray(wo, np.float32))
    res = _run(in_maps)
    out = np.zeros((BS, D), dtype=np.float32)
    for c in range(N_CORES):
        out += res.results[c]["out"]
    return out.reshape(B, S, D)


if __name__ == "__main__":
    t0 = time.time()
    _get_program()
    print(f"program build: {time.time() - t0:.1f}s")


# revision 21
# speedup vs baseline: 123.5262x; 123.5262x over previous
"""Multi-head attention (D=2048, H=16, B=2, S=2048, causal, RoPE) on 8 TRN2 cores.

Sharding: tensor-parallel over heads -- 2 heads per core, both batches.
Each core computes q/k/v projections for its 2 heads, RoPE, causal flash-style
attention, and a partial output projection over its heads' columns of wo.
The host sums the 8 partial outputs (the out-projection contracts over heads,
which is the sharded axis).

Self-contained: hardcodes all shapes; only needs numpy/ml_dtypes/concourse.
"""
import os
import sys
import time

for _p in ("/opt/trn_rl_repo",):
    if os.path.isdir(_p) and _p not in sys.path:
        sys.path.append(_p)

import numpy as np
import ml_dtypes
from contextlib import ExitStack

import concourse.bass as bass
import concourse.tile as tile
from concourse import bacc, mybir

BF = mybir.dt.bfloat16
F32 = mybir.dt.float32
BF_NP = ml_dtypes.bfloat16

B = 2
S = 2048
D = 2048
H = 16
HD = 128  # head dim
N_CORES = 8
H_CORE = H // N_CORES          # heads per core = 2
E = H_CORE * HD                # per-core q/k/v width = 256
BS = B * S                     # 4096 flattened tokens
P = 128
SC = 512                       # s-chunk (free dim of projection matmuls)
N_SC = BS // SC                # 8 s-chunks
N_DT = D // P                  # 16 d-tiles (contraction)
QC = 512                       # q-chunk in attention
N_QC = S // QC                 # 4 q-chunks per (batch, head)
N_KT = S // P                  # 16 k-tiles per (batch, head)
SCALE = 1.0 / float(np.sqrt(HD))
ROPE_BASE = 10000.0


def _build_program():
    """Build the per-core Bass program (identical on all cores; data differs)."""
    nc = bacc.Bacc("TRN2", target_bir_lowering=False, debug=False)

    xt_d = nc.dram_tensor("xt", [D, BS], BF, kind="ExternalInput").ap()
    wqt_d = nc.dram_tensor("wqt", [D, E], BF, kind="ExternalInput").ap()
    wkt_d = nc.dram_tensor("wkt", [D, E], BF, kind="ExternalInput").ap()
    wvt_d = nc.dram_tensor("wvt", [D, E], BF, kind="ExternalInput").ap()
    wot_d = nc.dram_tensor("wot", [E, D], BF, kind="ExternalInput").ap()
    cos_d = nc.dram_tensor("cos", [P, S], BF, kind="ExternalInput").ap()
    sin_d = nc.dram_tensor("sin", [P, S], BF, kind="ExternalInput").ap()
    rmat_d = nc.dram_tensor("rmat", [P, P], BF, kind="ExternalInput").ap()
    tri_d = nc.dram_tensor("tri", [P, P], BF, kind="ExternalInput").ap()
    out_d = nc.dram_tensor("out", [BS, D], F32, kind="ExternalOutput").ap()

    with tile.TileContext(nc) as tc:
        with ExitStack() as ctx:
            _emit(ctx, tc, nc, xt_d, wqt_d, wkt_d, wvt_d, wot_d,
                  cos_d, sin_d, rmat_d, tri_d, out_d)
    nc.compile()
    return nc


def _emit(ctx, tc, nc, xt_d, wqt_d, wkt_d, wvt_d, wot_d,
          cos_d, sin_d, rmat_d, tri_d, out_d):
    Exp = mybir.ActivationFunctionType.Exp

    const = ctx.enter_context(tc.tile_pool(name="const", bufs=1))
    xpool = ctx.enter_context(tc.tile_pool(name="xpool", bufs=2))
    qkv = ctx.enter_context(tc.tile_pool(name="qkv", bufs=1))
    rope = ctx.enter_context(tc.tile_pool(name="rope", bufs=4))
    att = ctx.enter_context(tc.tile_pool(name="att", bufs=6))
    nrm = ctx.enter_context(tc.tile_pool(name="nrm", bufs=3))
    outp = ctx.enter_context(tc.tile_pool(name="outp", bufs=4))
    psum = ctx.enter_context(tc.tile_pool(name="psum", bufs=8, space="PSUM"))

    # ---- constants / weights in SBUF ----
    def load_xt_chunk(sc, eng):
        xt_c = xpool.tile([P, N_DT * SC], BF, tag="xt")
        # one 3D-AP DMA for the whole 1 MB chunk (16 d-tiles x 512 cols)
        eng.dma_start(
            xt_c[:].rearrange("p (t s) -> p t s", s=SC),
            xt_d.rearrange("(t p) s -> p t s", p=P)[:, :, sc * SC:(sc + 1) * SC])
        return xt_c

    # prefetch the first x chunk before anything else so the first projection
    # matmuls aren't blocked behind the weight DMAs
    xt_c0 = load_xt_chunk(0, nc.sync)
    # w?t layout: [128 (d within tile), N_DT * E] bf16, d-tile major on free dim
    # (one 3D-AP DMA each, on the ACT queue; the SP queue carries x chunks)
    wq_sb = const.tile([P, N_DT * E], BF)
    wk_sb = const.tile([P, N_DT * E], BF)
    wv_sb = const.tile([P, N_DT * E], BF)
    for w_sb, w_d in ((wq_sb, wqt_d), (wk_sb, wkt_d), (wv_sb, wvt_d)):
        nc.scalar.dma_start(
            w_sb[:].rearrange("p (t e) -> p t e", e=E),
            w_d.rearrange("(t p) e -> p t e", p=P))
    cos_sb = const.tile([P, S], BF)
    sin_sb = const.tile([P, S], BF)
    nc.scalar.dma_start(cos_sb[:], cos_d[:])
    nc.scalar.dma_start(sin_sb[:], sin_d[:])
    rmat_sb = const.tile([P, P], BF)
    tri_sb = const.tile([P, P], BF)
    nc.scalar.dma_start(rmat_sb[:], rmat_d[:])
    nc.scalar.dma_start(tri_sb[:], tri_d[:])
    ones_sb = const.tile([P, P], BF)
    nc.vector.memset(ones_sb[:], 1.0)
    # wot: [E, D] -> [128, 2 * D], d'-tile (== head) major; needed only in
    # phase 3 so loaded last
    wo_sb = const.tile([P, H_CORE * D], BF)
    nc.scalar.dma_start(
        wo_sb[:].rearrange("p (t e) -> p t e", e=D),
        wot_d.rearrange("(t p) e -> p t e", p=P))

    # persistent activations
    qT = qkv.tile([P, H_CORE * BS], BF)   # [d, (head, b*s)] rope'd q
    kT = qkv.tile([P, H_CORE * BS], BF)   # [d, (head, b*s)] rope'd k
    v_sb = qkv.tile([P, (BS // P) * E], BF)  # [s within tile, (s-tile, e)]
    aoT = qkv.tile([P, H_CORE * BS], BF)  # [d, (b, head, q)] normalized attn out

    # ---- phase 1: projections + RoPE ----
    for sc in range(N_SC):
        b = sc // (N_SC // B)
        s_lo = (sc % (N_SC // B)) * SC  # within-batch s offset
        if sc == 0:
            xt_c = xt_c0
        else:
            xt_c = load_xt_chunk(sc, nc.sync)

        # qT / kT (with RoPE) per head (e-tile == head)
        for w_sb, dstT in ((wq_sb, qT), (wk_sb, kT)):
            for h in range(H_CORE):
                pp = psum.tile([P, SC], F32, tag="ps")
                for t in range(N_DT):
                    nc.tensor.matmul(
                        pp[:],
                        w_sb[:, t * E + h * HD: t * E + h * HD + HD],
                        xt_c[:, t * SC:(t + 1) * SC],
                        start=(t == 0), stop=(t == N_DT - 1))
                raw = rope.tile([P, SC], BF, tag="raw")
                nc.scalar.copy(raw[:], pp[:])
                rot = psum.tile([P, SC], F32, tag="ps")
                nc.tensor.matmul(rot[:], rmat_sb[:], raw[:], start=True, stop=True)
                t1 = rope.tile([P, SC], BF, tag="t1")
                nc.vector.tensor_mul(t1[:], raw[:], cos_sb[:, s_lo:s_lo + SC])
                t2 = rope.tile([P, SC], BF, tag="t2")
                nc.vector.tensor_mul(t2[:], rot[:], sin_sb[:, s_lo:s_lo + SC])
                dst = dstT[:, h * BS + sc * SC: h * BS + (sc + 1) * SC]
                nc.vector.tensor_add(dst, t1[:], t2[:])

        # v for this s-chunk: 4 s-subtiles of 128
        for st in range(SC // P):
            pv = psum.tile([P, SC], F32, tag="ps")
            for t in range(N_DT):
                nc.tensor.matmul(
                    pv[:, :E],
                    xt_c[:, t * SC + st * P: t * SC + (st + 1) * P],
                    wv_sb[:, t * E:(t + 1) * E],
                    start=(t == 0), stop=(t == N_DT - 1))
            g_st = sc * (SC // P) + st  # global s-tile index 0..31
            nc.scalar.copy(v_sb[:, g_st * E:(g_st + 1) * E], pv[:, :E])

    # ---- phase 2 + 3: attention per (batch, head); out-projection for each
    # q-chunk emitted as soon as both heads' attention output is ready, so the
    # 32 MB of output DMA overlaps the remaining attention compute ----
    def attention_chunk(b, h, qc):
        qk_off = h * BS + b * S  # column offset into qT/kT
        out_ps = psum.tile([P, QC], F32, tag="ps")
        den_ps = psum.tile([P, QC], F32, tag="ps")
        nkt = (qc + 1) * (QC // P)
        for j in range(nkt):
            di = j - qc * (QC // P)  # >=0 on diagonal tiles
            q0 = max(di, 0) * P      # valid q suffix start in chunk
            sc_ps = psum.tile([P, QC], F32, tag="ps")
            nc.tensor.matmul(
                sc_ps[:, q0:QC],
                kT[:, qk_off + j * P: qk_off + (j + 1) * P],
                qT[:, qk_off + qc * QC + q0: qk_off + (qc + 1) * QC],
                start=True, stop=True)
            at = att.tile([P, QC], BF, tag="at")
            nc.scalar.activation(at[:, q0:QC], sc_ps[:, q0:QC], Exp,
                                 scale=SCALE)
            if di >= 0:
                nc.vector.tensor_mul(at[:, q0:q0 + P],
                                     at[:, q0:q0 + P], tri_sb[:])
            g_st = b * (S // P) + j
            nc.tensor.matmul(
                out_ps[:, q0:QC],
                v_sb[:, g_st * E + h * HD: g_st * E + (h + 1) * HD],
                at[:, q0:QC],
                start=(j == 0), stop=(j == nkt - 1))
            nc.tensor.matmul(
                den_ps[:, q0:QC],
                ones_sb[:],
                at[:, q0:QC],
                start=(j == 0), stop=(j == nkt - 1))
        rec = nrm.tile([P, QC], F32, tag="rec")
        nc.vector.reciprocal_approx_fast(rec[:], den_ps[:])
        dst = aoT[:, (b * H_CORE + h) * S + qc * QC:
                  (b * H_CORE + h) * S + (qc + 1) * QC]
        nc.vector.tensor_mul(dst, out_ps[:], rec[:])

    def outproj_chunk(b, qc, tail=False):
        # rows [qc*QC, (qc+1)*QC) of batch b's output = 4 s-tiles of 128
        for st in range(qc * (QC // P), (qc + 1) * (QC // P)):
            for ec in range(D // SC):
                po = psum.tile([P, SC], F32, tag="ps")
                for h in range(H_CORE):
                    lhsT = aoT[:, (b * H_CORE + h) * S + st * P:
                               (b * H_CORE + h) * S + (st + 1) * P]
                    nc.tensor.matmul(
                        po[:],
                        lhsT,
                        wo_sb[:, h * D + ec * SC: h * D + (ec + 1) * SC],
                        start=(h == 0), stop=(h == H_CORE - 1))
                o_sb = outp.tile([P, SC], F32, tag="o")
                if tail and ec % 2 == 0:
                    # at the very end ACT is idle; share the eviction burst
                    nc.scalar.copy(o_sb[:], po[:])
                else:
                    nc.vector.tensor_copy(o_sb[:], po[:])
                nc.sync.dma_start(
                    out_d[b * S + st * P: b * S + (st + 1) * P,
                          ec * SC:(ec + 1) * SC],
                    o_sb[:])

    for b in range(B):
        for qc in range(N_QC):
            attention_chunk(b, 0, qc)
            prev = (b, qc - 1) if qc > 0 else (b - 1, N_QC - 1)
            if prev[0] >= 0:
                outproj_chunk(*prev)
            attention_chunk(b, 1, qc)
    outproj_chunk(B - 1, N_QC - 1, tail=True)


def _rope_tables():
    """cos/sin tables exactly matching the reference's indexing quirk."""
    inv_freq = (1.0 / (ROPE_BASE ** (np.arange(0, HD, 2, dtype=np.float32) / HD)))
    t = np.arange(S, dtype=np.float32)
    freqs = np.outer(t, inv_freq)                       # [S, 64]
    emb = np.concatenate([freqs, freqs], axis=1)        # [S, 128]
    cos_part = np.cos(emb)[:, ::2]                      # [S, 64]
    sin_part = np.sin(emb)[:, 1::2]                     # [S, 64]
    # COS[d, s] = cos_part[s, d // 2]
    cos = cos_part.T[np.repeat(np.arange(HD // 2), 2)]  # [128, S]
    sin = sin_part.T[np.repeat(np.arange(HD // 2), 2)]
    return np.ascontiguousarray(cos), np.ascontiguousarray(sin)


def _host_prep(x, wq, wk, wv, wo):
    """Build the per-core input maps."""
    bf = BF_NP
    xt = np.ascontiguousarray(
        x.reshape(BS, D).T.astype(bf))                  # [D, BS]
    cos, sin = _rope_tables()
    cos = cos.astype(bf)
    sin = sin.astype(bf)
    rmat = np.zeros((P, P), dtype=np.float32)           # R^T for rot = R @ q
    idx = np.arange(0, P, 2)
    rmat[idx + 1, idx] = -1.0                           # R^T[2j+1, 2j] = -1
    rmat[idx, idx + 1] = 1.0                            # R^T[2j, 2j+1] = +1
    rmat = rmat.astype(bf)
    tri = np.triu(np.ones((P, P), dtype=np.float32)).astype(bf)

    in_maps = []
    for c in range(N_CORES):
        lo, hi = c * E, (c + 1) * E
        in_maps.append({
            "xt": xt,
            "wqt": np.ascontiguousarray(wq[lo:hi].T.astype(bf)),
            "wkt": np.ascontiguousarray(wk[lo:hi].T.astype(bf)),
            "wvt": np.ascontiguousarray(wv[lo:hi].T.astype(bf)),
            "wot": np.ascontiguousarray(wo[:, lo:hi].T.astype(bf)),
            "cos": cos,
            "sin": sin,
            "rmat": rmat,
            "tri": tri,
        })
    return in_maps


_CACHE = {}


def _get_program():
    if "nc" not in _CACHE:
        _CACHE["nc"] = _build_program()
    return _CACHE["nc"]


def _run(in_maps):
    from concourse.bass_utils import run_bass_kernel_spmd
    nc = _get_program()
    res = run_bass_kernel_spmd(nc, in_maps, core_ids=list(range(N_CORES)))
    return res


def kernel(x, wq, wk, wv, wo, attn_mask=None, **_):
    x = np.asarray(x, dtype=np.float32)
    in_maps = _host_prep(np.asarray(x, np.float32), np.asarray(wq, np.float32),
                         np.asarray(wk, np.float32), np.asarray(wv, np.float32),
                         np.asarray(wo, np.float32))
    res = _run(in_maps)
    out = np.zeros((BS, D), dtype=np.float32)
    for c in range(N_CORES):
        out += res.results[c]["out"]
    return out.reshape(B, S, D)


if __name__ == "__main__":
    t0 = time.time()
    _get_program()
    print(f"program build: {time.time() - t0:.1f}s")
